# revision 1
# baseline (speedup 1.0000x reference)
"""Multi-head cross attention on 8 trn2 NeuronCores.

Problem: B=2, T=4096, EMB=512, H=8 heads (head dim 64), fp32 I/O.
  q = x1 @ Wq.T + bq ; k,v from x2 ; S = q k^T / sqrt(512) ;
  softmax over keys with -1e10 masking ; out = (A v) @ Wu.T + bu.

Sharding: core c handles batch b = c//4 and query rows
[1024*(c%4), 1024*(c%4+1)).  Each core computes K,V for its batch in
full (4-way duplication), its own Q chunk, attention, and out-proj.

Device-side layout choices:
  - All matmul operands fp16 (PE rate is dtype-independent; fp16 halves
    DMA/SBUF and keeps ~1e-3 accuracy), accumulation fp32 in PSUM.
  - Scores computed TRANSPOSED, S^T[key, query]: contraction over the
    head dim requires Q^T/K^T (head-dim on partitions), which fall out
    of computing the projections transposed from x^T inputs (host
    pre-transposes x1/x2/W).  With keys on partitions, P^T = exp(S^T)*M^T
    feeds the AV matmul directly as its stationary-side contraction
    without any on-chip transposes.
  - Scores are small (|S| < ~1) so exp needs no max-subtraction; the
    1/sqrt(512) scale is folded into the ACT exp instruction.
  - V is stored interleaved [key, head, 65] with a ones column so the
    AV matmul also produces the softmax denominators r[q] (row 64).
  - Normalization is deferred: Y^T_h / r_h via reciprocal + a K=1
    broadcast matmul + one DVE multiply per (head, chunk).
  - 2 heads are packed per scores pass via tile_position row-tiling
    (contraction=64 -> rows 0-63 / 64-127 run concurrently).
"""
import math
import os
from contextlib import ExitStack

import numpy as np

import concourse.bass as bass
import concourse.bacc as bacc
import concourse.tile as tile
import concourse.mybir as mybir
from concourse.bass_utils import run_bass_kernel_spmd

F16 = mybir.dt.float16
F32 = mybir.dt.float32
EXP = mybir.ActivationFunctionType.Exp

EMB, H, D, CT = 512, 8, 64, 4  # emb, heads, head dim, emb/128

FULL_CFG = dict(T=4096, QC=1024)  # keys per batch, query rows per core
MINI_CFG = dict(T=512, QC=256)


def attention_body(ctx, tc, io, cfg):
    nc = tc.nc
    T, QC = cfg["T"], cfg["QC"]
    KT = T // 128            # key tiles
    NG = KT // 2             # key-tile groups of 2
    CH = min(512, QC)        # query chunk width
    NCH = QC // CH
    scale = 1.0 / math.sqrt(EMB)

    pw = ctx.enter_context(tc.tile_pool(name="w", bufs=1))
    pk = ctx.enter_context(tc.tile_pool(name="kt", bufs=1))
    pv = ctx.enter_context(tc.tile_pool(name="v", bufs=1))
    pq = ctx.enter_context(tc.tile_pool(name="qt", bufs=1))

    # persistent weights / biases / constants
    wq = [pw.tile([128, EMB], F16, tag=f"wq{i}", name=f"wq{i}") for i in range(CT)]
    wk = [pw.tile([128, EMB], F16, tag=f"wk{i}", name=f"wk{i}") for i in range(CT)]
    wv = [pw.tile([128, EMB], F16, tag=f"wv{i}", name=f"wv{i}") for i in range(CT)]
    wu = [pw.tile([128, EMB], F16, tag=f"wu{i}", name=f"wu{i}") for i in range(CT)]
    for i in range(CT):
        nc.sync.dma_start(wq[i][:], io["wqT"][bass.ts(i, 128), :])
        nc.sync.dma_start(wk[i][:], io["wkT"][bass.ts(i, 128), :])
        nc.sync.dma_start(wv[i][:], io["wvT"][bass.ts(i, 128), :])
        nc.sync.dma_start(wu[i][:], io["wuT"][bass.ts(i, 128), :])
    bqr = pw.tile([128, CT], F32, tag="bqr", name="bqr")
    bkr = pw.tile([128, CT], F32, tag="bkr", name="bkr")
    bvb = pw.tile([128, EMB], F32, tag="bvb", name="bvb")
    bub = pw.tile([128, EMB], F32, tag="bub", name="bub")
    nc.sync.dma_start(bqr[:], io["bqr"][:, :])
    nc.sync.dma_start(bkr[:], io["bkr"][:, :])
    nc.sync.dma_start(bvb[:], io["bvb"][:, :])
    nc.sync.dma_start(bub[:], io["bub"][:, :])
    ones = pw.tile([1, D], F16, tag="ones", name="ones")
    nc.vector.memset(ones[:], 1.0)

    # persistent K^T [emb, T], V [T, head, 65(+pad)], Q^T [emb, QC]
    kt = [pk.tile([128, T], F16, tag=f"kt{i}", name=f"kt{i}") for i in range(CT)]
    v = pv.tile([128, KT, H, 66], F16, tag="v", name="v")
    nc.vector.memset(v[:, :, :, 64:65], 1.0)
    qt = [pq.tile([128, QC], F16, tag=f"qt{i}", name=f"qt{i}") for i in range(CT)]

    with tc.tile_pool(name="x", bufs=1) as px, \
         tc.tile_pool(name="pp", bufs=1, space="PSUM") as pp:
        x2t = [px.tile([128, T], F16, tag=f"x2t{i}", name=f"x2t{i}") for i in range(CT)]
        x1t = [px.tile([128, QC], F16, tag=f"x1t{i}", name=f"x1t{i}") for i in range(CT)]
        for i in range(CT):
            nc.sync.dma_start(x1t[i][:], io["x1T"][bass.ts(i, 128), :])
            for hf in range(2):
                nc.sync.dma_start(x2t[i][:, bass.ts(hf, T // 2)],
                                  io["x2T"][bass.ts(i, 128), bass.ts(hf, T // 2)])

        # Q^T[e,q] = sum_c WqT[c,e] * x1T[c,q]  (+ bq per-partition)
        for e in range(CT):
            for t in range(QC // CH):
                ps = pp.tile([128, CH], F32, tag=f"ps{t}", name="ps2")
                for c in range(CT):
                    nc.tensor.matmul(ps[:], wq[c][:, bass.ts(e, 128)],
                                     x1t[c][:, bass.ts(t, CH)],
                                     start=(c == 0), stop=(c == CT - 1))
                nc.vector.tensor_scalar_add(qt[e][:, bass.ts(t, CH)], ps[:],
                                            bqr[:, e:e + 1])
        # K^T[e,t] = sum_c WkT[c,e] * x2T[c,t] (+ bk); V[t,e] interleaved.
        # Stationary (wk) reused across 8 t-chunks per LDWEIGHTS.
        NT8 = min(8, T // 512)
        for e in range(CT):
            for tb in range(T // 512 // NT8):
                pss = [pp.tile([128, 512], F32, tag=f"ps{j}", name=f"ps{j}")
                       for j in range(NT8)]
                for c in range(CT):
                    for j in range(NT8):
                        nc.tensor.matmul(pss[j][:], wk[c][:, bass.ts(e, 128)],
                                         x2t[c][:, bass.ts(tb * NT8 + j, 512)],
                                         start=(c == 0), stop=(c == CT - 1))
                for j in range(NT8):
                    nc.vector.tensor_scalar_add(
                        kt[e][:, bass.ts(tb * NT8 + j, 512)], pss[j][:],
                        bkr[:, e:e + 1])
            if e > 0:
                continue
            # V right after K^T e-tile 0 so attention pair 0 can start
            for t in range(KT):
                ps = pp.tile([128, EMB], F32, tag="ps0", name="psv")
                for c in range(CT):
                    nc.tensor.matmul(ps[:], x2t[c][:, bass.ts(t, 128)], wv[c][:],
                                     start=(c == 0), stop=(c == CT - 1))
                nc.vector.tensor_add(
                    v[:, t, :, 0:64],
                    ps[:].rearrange("p (h d) -> p h d", h=H),
                    bvb[:].rearrange("p (h d) -> p h d", h=H))

    # optional debug dumps of intermediates
    if "dbg" in io:
        for e in range(CT):
            nc.sync.dma_start(io["dbg_qt"][bass.ts(e, 128), :], qt[e][:])
            nc.sync.dma_start(io["dbg_kt"][bass.ts(e, 128), :], kt[e][:])
        for t in range(KT):
            nc.sync.dma_start(
                io["dbg_v"][:, :].rearrange("p (a b) -> p a b", a=KT)[:, t, :],
                v[:, t, :, :].rearrange("p a b -> p (a b)"))

    # attention — both query chunks processed together so every matmul
    # stationary (kt slices, V tiles) is loaded once and reused, and exp
    # covers a full 4-bank PSUM span per key tile.
    QW = min(512, QC)        # matmul moving width (PSUM bank limit)
    NB = QC // QW            # query blocks
    with tc.tile_pool(name="ps_s", bufs=1, space="PSUM") as ps_s, \
         tc.tile_pool(name="ps_av", bufs=2, space="PSUM") as ps_av, \
         tc.tile_pool(name="pe", bufs=2) as pe, \
         tc.tile_pool(name="ppp", bufs=2) as ppp, \
         tc.tile_pool(name="pm", bufs=min(KT, 16)) as pm, \
         tc.tile_pool(name="py", bufs=CT) as py, \
         tc.tile_pool(name="pys", bufs=4) as pys, \
         tc.tile_pool(name="prr", bufs=2) as prr, \
         tc.tile_pool(name="po", bufs=2) as po:
        yts = [py.tile([128, QC], F16, tag="yt", name=f"yt{e}")
               for e in range(CT)]
        for pr in range(CT):  # head pair
            av = [ps_av.tile([65, QC], F32, tag="av", name="av")
                  for _ in range(2)]

            def mul_and_av(kk, e16, pt, mkt):
                nc.vector.tensor_mul(
                    pt[:].rearrange("p (c h q) -> p c h q", c=NB, h=2),
                    e16[:].rearrange("p (c h q) -> p c h q", c=NB, h=2),
                    mkt[:].rearrange("p (c q) -> p c q", c=NB)
                    .unsqueeze(2).broadcast_to([128, NB, 2, QW]))
                for hh in range(2):
                    for cb in range(NB):
                        nc.tensor.matmul(
                            av[hh][:, bass.ts(cb, QW)],
                            v[:, kk, 2 * pr + hh, 0:65],
                            pt[:, bass.ds((2 * cb + hh) * QW, QW)],
                            start=(kk == 0), stop=(kk == KT - 1))

            prev = None
            for kk in range(KT):
                mkt = pm.tile([128, QC], F16, tag="mk", name="mk")
                nc.sync.dma_start(mkt[:], io["maskT"][bass.ts(kk, 128), :])
                e16 = pe.tile([128, 2 * QC], F16, tag="E", name="e16")
                pt = ppp.tile([128, 2 * QC], F16, tag="P", name="pt")
                ps = ps_s.tile([128, 2 * QC], F32, tag="s", name="ps_s")
                for cb in range(NB):
                    for hh in range(2):  # head within pair
                        nc.tensor.matmul(
                            ps[:, bass.ds((2 * cb + hh) * QW, QW)],
                            kt[pr][bass.ds(64 * hh, 64), bass.ts(kk, 128)],
                            qt[pr][bass.ds(64 * hh, 64), bass.ts(cb, QW)],
                            start=True, stop=True,
                            tile_position=(64 * hh, 0))
                nc.scalar.activation(e16[:], ps[:], EXP, scale=scale)
                if prev is not None:
                    mul_and_av(*prev)
                prev = (kk, e16, pt, mkt)
            mul_and_av(*prev)

            # normalize: Y^T_h / r_h, r from the ones column (row 64)
            for hh in range(2):
                ysb = pys.tile([65, QC], F32, tag="ys", name="ysb")
                nc.scalar.copy(ysb[:], av[hh][:])
                r0 = prr.tile([1, QC], F32, tag="r0", name="r0")
                nc.vector.tensor_copy(r0[:], ysb[64:65, :])
                rr32 = prr.tile([1, QC], F32, tag="rr32", name="rr32")
                nc.vector.reciprocal_approx_fast(rr32[:], r0[:])
                rr = prr.tile([1, QC], F16, tag="rr", name="rr")
                with nc.allow_low_precision(reason="fp16 recip copy ok"):
                    nc.vector.tensor_copy(rr[:], rr32[:])
                bc = ps_av.tile([64, QC], F32, tag="av", name="bc")
                for cb in range(NB):
                    nc.tensor.matmul(bc[:, bass.ts(cb, QW)], ones[:],
                                     rr[:, bass.ts(cb, QW)],
                                     start=True, stop=True)
                nc.vector.tensor_mul(yts[pr][bass.ds(64 * hh, 64), :],
                                     ysb[0:64, :], bc[:])
        # out[q, :] = sum_e Y^T[e, q] * WuT[e, :] + bu
        for qi in range(QC // 128):
            pso = ps_av.tile([128, EMB], F32, tag="av", name="pso")
            for e in range(CT):
                nc.tensor.matmul(pso[:], yts[e][:, bass.ts(qi, 128)], wu[e][:],
                                 start=(e == 0), stop=(e == CT - 1))
            osb = po.tile([128, EMB], F32, tag="o", name="osb")
            nc.vector.tensor_add(osb[:], pso[:], bub[:])
            nc.sync.dma_start(io["out"][bass.ts(qi, 128), :], osb[:])


def build(cfg, num_devices=8, dbg=False):
    T, QC = cfg["T"], cfg["QC"]
    nc = bacc.Bacc("TRN2", target_bir_lowering=False, debug=False,
                   num_devices=num_devices)
    io = {
        "x1T": nc.dram_tensor("x1T", [EMB, QC], F16, kind="ExternalInput").ap(),
        "x2T": nc.dram_tensor("x2T", [EMB, T], F16, kind="ExternalInput").ap(),
        "maskT": nc.dram_tensor("maskT", [T, QC], F16, kind="ExternalInput").ap(),
        "wqT": nc.dram_tensor("wqT", [EMB, EMB], F16, kind="ExternalInput").ap(),
        "wkT": nc.dram_tensor("wkT", [EMB, EMB], F16, kind="ExternalInput").ap(),
        "wvT": nc.dram_tensor("wvT", [EMB, EMB], F16, kind="ExternalInput").ap(),
        "wuT": nc.dram_tensor("wuT", [EMB, EMB], F16, kind="ExternalInput").ap(),
        "bqr": nc.dram_tensor("bqr", [128, CT], F32, kind="ExternalInput").ap(),
        "bkr": nc.dram_tensor("bkr", [128, CT], F32, kind="ExternalInput").ap(),
        "bvb": nc.dram_tensor("bvb", [128, EMB], F32, kind="ExternalInput").ap(),
        "bub": nc.dram_tensor("bub", [128, EMB], F32, kind="ExternalInput").ap(),
        "out": nc.dram_tensor("out", [QC, EMB], F32, kind="ExternalOutput").ap(),
    }
    if dbg:
        io["dbg"] = True
        CH = min(512, QC)
        io["dbg_qt"] = nc.dram_tensor("dbg_qt", [EMB, QC], F16, kind="ExternalOutput").ap()
        io["dbg_kt"] = nc.dram_tensor("dbg_kt", [EMB, T], F16, kind="ExternalOutput").ap()
        io["dbg_v"] = nc.dram_tensor("dbg_v", [128, (T // 128) * H * 66], F16, kind="ExternalOutput").ap()
        io["dbg_e"] = nc.dram_tensor("dbg_e", [128, 2 * CH], F16, kind="ExternalOutput").ap()
        io["dbg_p"] = nc.dram_tensor("dbg_p", [128, 2 * CH], F16, kind="ExternalOutput").ap()
        io["dbg_y"] = nc.dram_tensor("dbg_y", [65, CH], F32, kind="ExternalOutput").ap()
    with tile.TileContext(nc) as tc:
        with ExitStack() as ctx:
            attention_body(ctx, tc, io, cfg)
    nc.compile()
    return nc


def host_prep(x1, x2, mask, Wq, bq, Wk, bk, Wv, bv, Wu, bu, cfg):
    """Build the 8 per-core input maps from full inputs."""
    T, QC = cfg["T"], cfg["QC"]
    shared = {
        "wqT": np.ascontiguousarray(Wq.T).astype(np.float16),
        "wkT": np.ascontiguousarray(Wk.T).astype(np.float16),
        "wvT": np.ascontiguousarray(Wv.T).astype(np.float16),
        "wuT": np.ascontiguousarray(Wu.T).astype(np.float16),
        "bqr": np.ascontiguousarray(bq.reshape(CT, 128).T).astype(np.float32),
        "bkr": np.ascontiguousarray(bk.reshape(CT, 128).T).astype(np.float32),
        "bvb": np.ascontiguousarray(np.broadcast_to(bv, (128, EMB))).astype(np.float32),
        "bub": np.ascontiguousarray(np.broadcast_to(bu, (128, EMB))).astype(np.float32),
    }
    x2T = [x2[b].T.astype(np.float16) for b in range(x1.shape[0])]
    in_maps = []
    n_cores = (x1.shape[0] * x1.shape[1]) // QC
    per_b = x1.shape[1] // QC
    for c in range(n_cores):
        b, q0 = c // per_b, (c % per_b) * QC
        in_maps.append(dict(
            shared,
            x1T=x1[b, q0:q0 + QC, :].T.astype(np.float16),
            x2T=x2T[b],
            maskT=mask[b, q0:q0 + QC, :].T.astype(np.float16),
        ))
    return in_maps


_NC_CACHE = {}


def kernel(x1, x2, mask, Wq, bq, Wk, bk, Wv, bv, Wu, bu):
    cfg = FULL_CFG
    B, TQ, _ = x1.shape
    in_maps = host_prep(np.asarray(x1, np.float32), np.asarray(x2, np.float32),
                        np.asarray(mask), np.asarray(Wq, np.float32),
                        np.asarray(bq, np.float32), np.asarray(Wk, np.float32),
                        np.asarray(bk, np.float32), np.asarray(Wv, np.float32),
                        np.asarray(bv, np.float32), np.asarray(Wu, np.float32),
                        np.asarray(bu, np.float32), cfg)
    key = (cfg["T"], cfg["QC"])
    if key not in _NC_CACHE:
        _NC_CACHE[key] = build(cfg)
    nc = _NC_CACHE[key]
    res = run_bass_kernel_spmd(nc, in_maps, core_ids=list(range(8)),
                               trace=bool(os.environ.get("KERNEL_TRACE")))
    if os.environ.get("KERNEL_TRACE"):
        kernel.last_exec_ns = res.exec_time_ns
        kernel.last_results = res
    out = np.empty((B, TQ, EMB), np.float32)
    per_b = TQ // cfg["QC"]
    for c in range(8):
        b, q0 = c // per_b, (c % per_b) * cfg["QC"]
        out[b, q0:q0 + cfg["QC"], :] = res.results[c]["out"]
    return out



# revision 9
# speedup vs baseline: 1.1409x; 1.1409x over previous
"""Multi-head cross attention on 8 trn2 NeuronCores.

Problem: B=2, T=4096, EMB=512, H=8 heads (head dim 64), fp32 I/O.
  q = x1 @ Wq.T + bq ; k,v from x2 ; S = q k^T / sqrt(512) ;
  softmax over keys with -1e10 masking ; out = (A v) @ Wu.T + bu.

Sharding: core c handles batch b = c//4 and query rows
[1024*(c%4), 1024*(c%4+1)).  Each core computes K,V for its batch in
full (4-way duplication), its own Q chunk, attention, and out-proj.

Schedule (v2): the kernel is ACT-bound (exp of 33.5M scores/core at
~1.2GHz*128 lanes ~= 245us).  Everything is organized so the scalar
engine streams exp back-to-back:
  - Attention runs in (head-pair, query-chunk-of-512, key-tile) units.
    Score PSUM tiles are [128, 2*512] (2 banks) double-buffered, so
    scores(u+1) runs on PE while exp(u) drains on ACT - no ping-pong.
  - AV accumulators [65, 512] are 1 PSUM bank each (ones column gives
    softmax denominators), 4-slot ring. 4+4 banks total.
  - mask is DMA'd ONCE (24 tiles persistent + small ring re-fetched
    per pr) instead of per head-pair: 32MiB -> ~11MiB HBM traffic.
  - K^T (e>0) and V projections are interleaved into pr=0's units so
    the PE fills exp-latency gaps instead of running a serial prelude.
  - V-assembly bias-adds run on GpSimd (idle engine) to keep DVE
    (mask multiply, 2x fp16 mode) off the critical path.
"""
import math
import os
from contextlib import ExitStack

import numpy as np

import concourse.bass as bass
import concourse.bacc as bacc
import concourse.tile as tile
import concourse.mybir as mybir
from concourse.bass_utils import run_bass_kernel_spmd

F16 = mybir.dt.float16
F32 = mybir.dt.float32
EXP = mybir.ActivationFunctionType.Exp

EMB, H, D, CT = 512, 8, 64, 4  # emb, heads, head dim, emb/128
PR = H // 2                    # head pairs

FULL_CFG = dict(T=4096, QC=1024)  # keys per batch, query rows per core
MINI_CFG = dict(T=512, QC=256)

V_ADD_ON_GPSIMD = False  # walrus: "GPSIMD Instructions cannot access PSUM"


def attention_body(ctx, tc, io, cfg):
    nc = tc.nc
    T, QC = cfg["T"], cfg["QC"]
    KT = T // 128            # key tiles
    CH = min(512, QC)        # query chunk width (1 PSUM bank per AV acc)
    NCH = QC // CH
    MH = min(20, KT)         # mask tiles held persistently in SBUF
    scale = 1.0 / math.sqrt(EMB)

    pw = ctx.enter_context(tc.tile_pool(name="w", bufs=1))
    pk = ctx.enter_context(tc.tile_pool(name="kt", bufs=1))
    pv = ctx.enter_context(tc.tile_pool(name="v", bufs=1))
    pq = ctx.enter_context(tc.tile_pool(name="qt", bufs=1))
    pm = ctx.enter_context(tc.tile_pool(name="mk", bufs=1))
    pmr = ctx.enter_context(tc.tile_pool(name="mkr", bufs=4))
    px2 = ctx.enter_context(tc.tile_pool(name="x2", bufs=1))
    # PSUM: ps = [128, 2*CH] fp32 tiles (2 banks) x2; pav = [128, CH]
    # long-lived AV accumulators (1 bank) x2; pvp = transient 1-bank
    # tiles (V-proj psum, bc broadcast, out-proj) x2.  8 banks total.
    ps = ctx.enter_context(tc.tile_pool(name="ps", bufs=2, space="PSUM"))
    pav = ctx.enter_context(tc.tile_pool(name="pav", bufs=2, space="PSUM"))
    pvp = ctx.enter_context(tc.tile_pool(name="pvp", bufs=2, space="PSUM"))

    # persistent weights / biases / constants
    wq = [pw.tile([128, EMB], F16, tag=f"wq{i}", name=f"wq{i}") for i in range(CT)]
    wk = [pw.tile([128, EMB], F16, tag=f"wk{i}", name=f"wk{i}") for i in range(CT)]
    wv = [pw.tile([128, EMB], F16, tag=f"wv{i}", name=f"wv{i}") for i in range(CT)]
    wu = [pw.tile([128, EMB], F16, tag=f"wu{i}", name=f"wu{i}") for i in range(CT)]
    for i in range(CT):
        nc.sync.dma_start(wq[i][:], io["wqT"][bass.ts(i, 128), :])
        nc.sync.dma_start(wk[i][:], io["wkT"][bass.ts(i, 128), :])
        nc.sync.dma_start(wv[i][:], io["wvT"][bass.ts(i, 128), :])
        nc.sync.dma_start(wu[i][:], io["wuT"][bass.ts(i, 128), :])
    bqr = pw.tile([128, CT], F32, tag="bqr", name="bqr")
    bkr = pw.tile([128, CT], F32, tag="bkr", name="bkr")
    bvb = pw.tile([128, EMB], F32, tag="bvb", name="bvb")
    bub = pw.tile([128, EMB], F32, tag="bub", name="bub")
    nc.sync.dma_start(bqr[:], io["bqr"][:, :])
    nc.sync.dma_start(bkr[:], io["bkr"][:, :])
    nc.sync.dma_start(bvb[:], io["bvb"][:, :])
    nc.sync.dma_start(bub[:], io["bub"][:, :])
    ones = pw.tile([1, D], F16, tag="ones", name="ones")
    nc.vector.memset(ones[:], 1.0)

    # persistent K^T [emb, T], V [key, head, 65(+pad)], Q^T [emb, QC]
    kt = [pk.tile([128, T], F16, tag=f"kt{i}", name=f"kt{i}") for i in range(CT)]
    v = pv.tile([128, KT, H, 66], F16, tag="v", name="v")
    nc.vector.memset(v[:, :, :, 64:65], 1.0)
    qt = [pq.tile([128, QC], F16, tag=f"qt{i}", name=f"qt{i}") for i in range(CT)]

    # x inputs; x2 stays resident until the interleaved K/V feed drains
    x2t = [px2.tile([128, T], F16, tag=f"x2t{i}", name=f"x2t{i}") for i in range(CT)]
    for i in range(CT):
        for hf in range(2):
            nc.sync.dma_start(x2t[i][:, bass.ts(hf, T // 2)],
                              io["x2T"][bass.ts(i, 128), bass.ts(hf, T // 2)])

    # mask tiles: DMA'd once; first MH persist, the rest re-fetched per pr
    mks = [pm.tile([128, QC], F16, tag=f"mk{k}", name=f"mk{k}")
           for k in range(MH)]
    for k in range(MH):
        nc.sync.dma_start(mks[k][:], io["maskT"][bass.ts(k, 128), :])

    def mask_tile(pr, kk):
        if kk < MH:
            return mks[kk]
        mt = pmr.tile([128, QC], F16, tag="mr", name=f"mr{pr}_{kk}")
        nc.sync.dma_start(mt[:], io["maskT"][bass.ts(kk, 128), :])
        return mt

    # ---- projections -------------------------------------------------
    def emit_k(e, tb):  # one [128, QC] tile of K^T for e-slice e
        kp = ps.tile([128, 2 * CH], F32, tag="s", name=f"kp{e}_{tb}")
        for sub in range(QC // CH):
            for c in range(CT):
                nc.tensor.matmul(kp[:, bass.ts(sub, CH)],
                                 wk[c][:, bass.ts(e, 128)],
                                 x2t[c][:, bass.ds(tb * QC + sub * CH, CH)],
                                 start=(c == 0), stop=(c == CT - 1))
        nc.vector.tensor_scalar_add(kt[e][:, bass.ts(tb, QC)], kp[:, 0:QC],
                                    bkr[:, e:e + 1])

    def emit_v(t):  # one key-tile of V, interleaved layout + ones col
        vp = pvp.tile([128, CH], F32, tag="vp", name=f"vp{t}")
        for c in range(CT):
            nc.tensor.matmul(vp[:, 0:EMB], x2t[c][:, bass.ts(t, 128)], wv[c][:],
                             start=(c == 0), stop=(c == CT - 1))
        eng = nc.gpsimd if V_ADD_ON_GPSIMD else nc.vector
        eng.tensor_add(
            v[:, t, :, 0:64],
            vp[:, 0:EMB].rearrange("p (h d) -> p h d", h=H),
            bvb[:].rearrange("p (h d) -> p h d", h=H))

    with tc.tile_pool(name="x1", bufs=1) as px1:
        x1t = [px1.tile([128, QC], F16, tag=f"x1t{i}", name=f"x1t{i}")
               for i in range(CT)]
        for i in range(CT):
            nc.sync.dma_start(x1t[i][:], io["x1T"][bass.ts(i, 128), :])
        # Q^T[e,q] = sum_c WqT[c,e] * x1T[c,q]  (+ bq per-partition)
        for e in range(CT):
            qp = ps.tile([128, 2 * CH], F32, tag="s", name=f"qp{e}")
            for sub in range(QC // CH):
                for c in range(CT):
                    nc.tensor.matmul(qp[:, bass.ts(sub, CH)],
                                     wq[c][:, bass.ts(e, 128)],
                                     x1t[c][:, bass.ts(sub, CH)],
                                     start=(c == 0), stop=(c == CT - 1))
            nc.vector.tensor_scalar_add(qt[e][:, 0:QC], qp[:, 0:QC],
                                        bqr[:, e:e + 1])

    # upfront: K^T e=0 (scores pr=0 stationary) and V key-tile 0
    for tb in range(T // QC):
        emit_k(0, tb)
    emit_v(0)
    # deferred: remaining V tiles ride the pr=0/ch=0 unit loop; K e=1..3
    # tiles are pumped during later pr=0 units.
    feed = [(lambda e=e, tb=tb: emit_k(e, tb))
            for e in range(1, CT) for tb in range(T // QC)]

    # ---- attention ---------------------------------------------------
    pe_ = ctx.enter_context(tc.tile_pool(name="pe", bufs=2))
    ppt = ctx.enter_context(tc.tile_pool(name="ppt", bufs=2))
    prr = ctx.enter_context(tc.tile_pool(name="prr", bufs=2))
    py = ctx.enter_context(tc.tile_pool(name="py", bufs=1))
    po = ctx.enter_context(tc.tile_pool(name="po", bufs=2))
    yts = [py.tile([128, QC], F16, tag=f"yt{e}", name=f"yt{e}")
           for e in range(CT)]

    for pr in range(PR):
        for ch in range(NCH):
            av = [pav.tile([128, CH], F32, tag="av", name=f"av{pr}_{ch}{hh}")
                  for hh in range(2)]
            for kk in range(KT):
                mkt = mask_tile(pr, kk)
                s = ps.tile([128, 2 * CH], F32, tag="s", name="s")
                for hh in range(2):
                    nc.tensor.matmul(s[:, bass.ds(hh * CH, CH)],
                                     kt[pr][bass.ds(64 * hh, 64), bass.ts(kk, 128)],
                                     qt[pr][bass.ds(64 * hh, 64), bass.ds(ch * CH, CH)],
                                     start=True, stop=True,
                                     tile_position=(64 * hh, 0))
                # PE fills exp/mult latency with projection feed work
                if pr == 0:
                    if ch == 0 and kk + 1 < KT:
                        emit_v(kk + 1)
                    elif (ch > 0 or kk + 1 >= KT) and feed and kk % 3 == 0:
                        feed.pop(0)()
                e16 = pe_.tile([128, 2 * CH], F16, tag="E", name="e16")
                nc.scalar.activation(e16[:], s[:], EXP, scale=scale)
                pt = ppt.tile([128, 2 * CH], F16, tag="P", name="pt")
                nc.vector.tensor_mul(
                    pt[:].rearrange("p (h q) -> p h q", h=2),
                    e16[:].rearrange("p (h q) -> p h q", h=2),
                    mkt[:, bass.ds(ch * CH, CH)].unsqueeze(1)
                    .broadcast_to([128, 2, CH]))
                for hh in range(2):
                    nc.tensor.matmul(av[hh][0:65, :], v[:, kk, 2 * pr + hh, 0:65],
                                     pt[:, bass.ds(hh * CH, CH)],
                                     start=(kk == 0), stop=(kk == KT - 1))
            if pr == 0 and ch == NCH - 1:
                while feed:
                    feed.pop(0)()
            # normalize: Y^T_h / r_h, r from the ones column (row 64)
            for hh in range(2):
                r0 = prr.tile([1, CH], F32, tag="r0", name="r0")
                nc.vector.tensor_copy(r0[:], av[hh][64:65, :])
                rr32 = prr.tile([1, CH], F32, tag="rr32", name="rr32")
                nc.vector.reciprocal_approx_fast(rr32[:], r0[:])
                rr = prr.tile([1, CH], F16, tag="rr", name="rr")
                with nc.allow_low_precision(reason="fp16 recip copy ok"):
                    nc.vector.tensor_copy(rr[:], rr32[:])
                bc = pvp.tile([128, CH], F32, tag="vp", name=f"bc{pr}_{ch}{hh}")
                nc.tensor.matmul(bc[0:64, :], ones[:], rr[:],
                                 start=True, stop=True)
                # DVE may read only one PSUM operand: stage av in SBUF fp16
                ysb = prr.tile([64, CH], F16, tag="ysb", name="ysb")
                with nc.allow_low_precision(reason="y fp16 staging ok"):
                    nc.vector.tensor_copy(ysb[:], av[hh][0:64, :])
                nc.vector.tensor_mul(
                    yts[pr][bass.ds(64 * hh, 64), bass.ds(ch * CH, CH)],
                    ysb[:], bc[0:64, :])

    # out[q, :] = sum_e Y^T[e, q] * WuT[e, :] + bu
    for qi in range(QC // 128):
        pso = pvp.tile([128, CH], F32, tag="vp", name=f"pso{qi}")
        for e in range(CT):
            nc.tensor.matmul(pso[:, 0:EMB], yts[e][:, bass.ts(qi, 128)], wu[e][:],
                             start=(e == 0), stop=(e == CT - 1))
        osb = po.tile([128, EMB], F32, tag="o", name="osb")
        nc.vector.tensor_add(osb[:], pso[:, 0:EMB], bub[:])
        nc.sync.dma_start(io["out"][bass.ts(qi, 128), :], osb[:])


def build(cfg, num_devices=8):
    T, QC = cfg["T"], cfg["QC"]
    nc = bacc.Bacc("TRN2", target_bir_lowering=False, debug=False,
                   num_devices=num_devices)
    io = {
        "x1T": nc.dram_tensor("x1T", [EMB, QC], F16, kind="ExternalInput").ap(),
        "x2T": nc.dram_tensor("x2T", [EMB, T], F16, kind="ExternalInput").ap(),
        "maskT": nc.dram_tensor("maskT", [T, QC], F16, kind="ExternalInput").ap(),
        "wqT": nc.dram_tensor("wqT", [EMB, EMB], F16, kind="ExternalInput").ap(),
        "wkT": nc.dram_tensor("wkT", [EMB, EMB], F16, kind="ExternalInput").ap(),
        "wvT": nc.dram_tensor("wvT", [EMB, EMB], F16, kind="ExternalInput").ap(),
        "wuT": nc.dram_tensor("wuT", [EMB, EMB], F16, kind="ExternalInput").ap(),
        "bqr": nc.dram_tensor("bqr", [128, CT], F32, kind="ExternalInput").ap(),
        "bkr": nc.dram_tensor("bkr", [128, CT], F32, kind="ExternalInput").ap(),
        "bvb": nc.dram_tensor("bvb", [128, EMB], F32, kind="ExternalInput").ap(),
        "bub": nc.dram_tensor("bub", [128, EMB], F32, kind="ExternalInput").ap(),
        "out": nc.dram_tensor("out", [QC, EMB], F32, kind="ExternalOutput").ap(),
    }
    with tile.TileContext(nc) as tc:
        with ExitStack() as ctx:
            attention_body(ctx, tc, io, cfg)
    nc.compile()
    return nc


def host_prep(x1, x2, mask, Wq, bq, Wk, bk, Wv, bv, Wu, bu, cfg):
    """Build the 8 per-core input maps from full inputs."""
    T, QC = cfg["T"], cfg["QC"]
    shared = {
        "wqT": np.ascontiguousarray(Wq.T).astype(np.float16),
        "wkT": np.ascontiguousarray(Wk.T).astype(np.float16),
        "wvT": np.ascontiguousarray(Wv.T).astype(np.float16),
        "wuT": np.ascontiguousarray(Wu.T).astype(np.float16),
        "bqr": np.ascontiguousarray(bq.reshape(CT, 128).T).astype(np.float32),
        "bkr": np.ascontiguousarray(bk.reshape(CT, 128).T).astype(np.float32),
        "bvb": np.ascontiguousarray(np.broadcast_to(bv, (128, EMB))).astype(np.float32),
        "bub": np.ascontiguousarray(np.broadcast_to(bu, (128, EMB))).astype(np.float32),
    }
    x2T = [x2[b].T.astype(np.float16) for b in range(x1.shape[0])]
    in_maps = []
    n_cores = (x1.shape[0] * x1.shape[1]) // QC
    per_b = x1.shape[1] // QC
    for c in range(n_cores):
        b, q0 = c // per_b, (c % per_b) * QC
        in_maps.append(dict(
            shared,
            x1T=x1[b, q0:q0 + QC, :].T.astype(np.float16),
            x2T=x2T[b],
            maskT=mask[b, q0:q0 + QC, :].T.astype(np.float16),
        ))
    return in_maps


_NC_CACHE = {}


def kernel(x1, x2, mask, Wq, bq, Wk, bk, Wv, bv, Wu, bu):
    cfg = FULL_CFG
    B, TQ, _ = x1.shape
    in_maps = host_prep(np.asarray(x1, np.float32), np.asarray(x2, np.float32),
                        np.asarray(mask), np.asarray(Wq, np.float32),
                        np.asarray(bq, np.float32), np.asarray(Wk, np.float32),
                        np.asarray(bk, np.float32), np.asarray(Wv, np.float32),
                        np.asarray(bv, np.float32), np.asarray(Wu, np.float32),
                        np.asarray(bu, np.float32), cfg)
    key = (cfg["T"], cfg["QC"])
    if key not in _NC_CACHE:
        _NC_CACHE[key] = build(cfg)
    nc = _NC_CACHE[key]
    res = run_bass_kernel_spmd(nc, in_maps, core_ids=list(range(8)),
                               trace=bool(os.environ.get("KERNEL_TRACE")))
    if os.environ.get("KERNEL_TRACE"):
        kernel.last_exec_ns = res.exec_time_ns
        kernel.last_results = res
    out = np.empty((B, TQ, EMB), np.float32)
    per_b = TQ // cfg["QC"]
    for c in range(8):
        b, q0 = c // per_b, (c % per_b) * cfg["QC"]
        out[b, q0:q0 + cfg["QC"], :] = res.results[c]["out"]
    return out


# revision 15
# speedup vs baseline: 1.2067x; 1.0577x over previous
"""Multi-head cross attention on 8 trn2 NeuronCores.

Problem: B=2, T=4096, EMB=512, H=8 heads (head dim 64), fp32 I/O.
  q = x1 @ Wq.T + bq ; k,v from x2 ; S = q k^T / sqrt(512) ;
  softmax over keys with -1e10 masking ; out = (A v) @ Wu.T + bu.

Sharding: core c handles batch b = c//4 and query rows
[1024*(c%4), 1024*(c%4+1)).  Each core computes K,V for its batch in
full (4-way duplication), its own Q chunk, attention, and out-proj.

Schedule (v2): the kernel is ACT-bound (exp of 33.5M scores/core at
~1.2GHz*128 lanes ~= 245us).  Everything is organized so the scalar
engine streams exp back-to-back:
  - Attention runs in (head-pair, query-chunk-of-512, key-tile) units.
    Score PSUM tiles are [128, 2*512] (2 banks) double-buffered, so
    scores(u+1) runs on PE while exp(u) drains on ACT - no ping-pong.
  - AV accumulators [65, 512] are 1 PSUM bank each (ones column gives
    softmax denominators), 4-slot ring. 4+4 banks total.
  - mask is DMA'd ONCE (24 tiles persistent + small ring re-fetched
    per pr) instead of per head-pair: 32MiB -> ~11MiB HBM traffic.
  - K^T (e>0) and V projections are interleaved into pr=0's units so
    the PE fills exp-latency gaps instead of running a serial prelude.
  - V-assembly bias-adds run on GpSimd (idle engine) to keep DVE
    (mask multiply, 2x fp16 mode) off the critical path.
"""
import math
import os
from contextlib import ExitStack

import numpy as np

import concourse.bass as bass
import concourse.bacc as bacc
import concourse.tile as tile
import concourse.mybir as mybir
from concourse.bass_utils import run_bass_kernel_spmd

F16 = mybir.dt.float16
F32 = mybir.dt.float32
EXP = mybir.ActivationFunctionType.Exp

EMB, H, D, CT = 512, 8, 64, 4  # emb, heads, head dim, emb/128
PR = H // 2                    # head pairs

FULL_CFG = dict(T=4096, QC=1024)  # keys per batch, query rows per core
MINI_CFG = dict(T=512, QC=256)

V_ADD_ON_GPSIMD = False  # walrus: "GPSIMD Instructions cannot access PSUM"


def attention_body(ctx, tc, io, cfg):
    nc = tc.nc
    T, QC = cfg["T"], cfg["QC"]
    KT = T // 128            # key tiles
    CH = min(512, QC)        # query chunk width (1 PSUM bank per AV acc)
    NCH = QC // CH
    MH = min(18, KT)         # mask tiles held persistently in SBUF
    scale = 1.0 / math.sqrt(EMB)

    pw = ctx.enter_context(tc.tile_pool(name="w", bufs=1))
    pk = ctx.enter_context(tc.tile_pool(name="kt", bufs=1))
    pv = ctx.enter_context(tc.tile_pool(name="v", bufs=1))
    pq = ctx.enter_context(tc.tile_pool(name="qt", bufs=1))
    pm = ctx.enter_context(tc.tile_pool(name="mk", bufs=1))
    pmr = ctx.enter_context(tc.tile_pool(name="mkr", bufs=6))
    px2 = ctx.enter_context(tc.tile_pool(name="x2", bufs=1))
    # PSUM: ps = [128, 2*CH] fp32 tiles (2 banks) x2; pav = [128, CH]
    # long-lived AV accumulators (1 bank) x2; pvp = transient 1-bank
    # tiles (V-proj psum, bc broadcast, out-proj) x2.  8 banks total.
    ps = ctx.enter_context(tc.tile_pool(name="ps", bufs=2, space="PSUM"))
    pav = ctx.enter_context(tc.tile_pool(name="pav", bufs=2, space="PSUM"))
    pvp = ctx.enter_context(tc.tile_pool(name="pvp", bufs=2, space="PSUM"))

    # persistent weights / biases / constants.  DMA issue order is the
    # critical-path order: x1+wq (Q proj) first, then x2 halves + wk
    # (K^T e=0), wv, then the bulk mask tiles.  The DMA queue is FIFO,
    # so putting mask first would delay the first exp by ~50us.
    wq = [pw.tile([128, EMB], F16, tag=f"wq{i}", name=f"wq{i}") for i in range(CT)]
    wk = [pw.tile([128, EMB], F16, tag=f"wk{i}", name=f"wk{i}") for i in range(CT)]
    wv = [pw.tile([128, EMB], F16, tag=f"wv{i}", name=f"wv{i}") for i in range(CT)]
    wu = [pw.tile([128, EMB], F16, tag=f"wu{i}", name=f"wu{i}") for i in range(CT)]
    bqr = pw.tile([128, CT], F32, tag="bqr", name="bqr")
    bkr = pw.tile([128, CT], F32, tag="bkr", name="bkr")
    bvb = pw.tile([128, EMB], F32, tag="bvb", name="bvb")
    bub = pw.tile([128, EMB], F32, tag="bub", name="bub")
    ones = pw.tile([1, D], F16, tag="ones", name="ones")
    nc.vector.memset(ones[:], 1.0)

    # persistent K^T [emb, T], V [key, head, 65(+pad)], Q^T [emb, QC]
    kt = [pk.tile([128, T], F16, tag=f"kt{i}", name=f"kt{i}") for i in range(CT)]
    v = pv.tile([128, KT, H, 66], F16, tag="v", name="v")
    nc.vector.memset(v[:, :, :, 64:65], 1.0)
    qt = [pq.tile([128, QC], F16, tag=f"qt{i}", name=f"qt{i}") for i in range(CT)]

    x2t = [px2.tile([128, T], F16, tag=f"x2t{i}", name=f"x2t{i}") for i in range(CT)]
    px1 = ctx.enter_context(tc.tile_pool(name="x1", bufs=1))
    x1t = [px1.tile([128, QC], F16, tag=f"x1t{i}", name=f"x1t{i}")
           for i in range(CT)]
    for i in range(CT):
        nc.sync.dma_start(x1t[i][:], io["x1T"][bass.ts(i, 128), :])
    for i in range(CT):
        nc.sync.dma_start(wq[i][:], io["wqT"][bass.ts(i, 128), :])
    nc.sync.dma_start(bqr[:], io["bqr"][:, :])
    for i in range(CT):  # x2 low halves first: K^T e=0 tb=0,1 unblocks
        nc.sync.dma_start(x2t[i][:, bass.ts(0, T // 2)],
                          io["x2T"][bass.ts(i, 128), bass.ts(0, T // 2)])
    for i in range(CT):
        nc.sync.dma_start(wk[i][:], io["wkT"][bass.ts(i, 128), :])
        nc.sync.dma_start(wv[i][:], io["wvT"][bass.ts(i, 128), :])
    nc.sync.dma_start(bkr[:], io["bkr"][:, :])
    nc.sync.dma_start(bvb[:], io["bvb"][:, :])
    for i in range(CT):
        nc.sync.dma_start(x2t[i][:, bass.ts(1, T // 2)],
                          io["x2T"][bass.ts(i, 128), bass.ts(1, T // 2)])
    for i in range(CT):
        nc.sync.dma_start(wu[i][:], io["wuT"][bass.ts(i, 128), :])
    nc.sync.dma_start(bub[:], io["bub"][:, :])

    # mask tiles: first MH persist (fetched once); the tail KT-MH are
    # re-fetched per (pr, ch) as half-width ring tiles, prefetched a few
    # units ahead so the mask multiply never waits on DMA.
    mks = [pm.tile([128, QC], F16, tag=f"mk{k}", name=f"mk{k}")
           for k in range(MH)]
    for k in range(MH):
        nc.sync.dma_start(mks[k][:], io["maskT"][bass.ts(k, 128), :])

    mring = {}

    def mask_prefetch(pr, ch, kk):
        if kk < MH or kk >= KT or (pr, ch, kk) in mring:
            return
        mt = pmr.tile([128, CH], F16, tag="mr", name=f"mr{pr}_{ch}_{kk}")
        nc.sync.dma_start(mt[:], io["maskT"][bass.ts(kk, 128),
                                             bass.ds(ch * CH, CH)])
        mring[(pr, ch, kk)] = mt

    def mask_ap(pr, ch, kk):  # [128, CH] slice for this unit
        if kk < MH:
            return mks[kk][:, bass.ds(ch * CH, CH)]
        return mring.pop((pr, ch, kk))[:]

    # ---- projections -------------------------------------------------
    def emit_k(e, tb):  # one [128, QC] tile of K^T for e-slice e
        kp = ps.tile([128, 2 * CH], F32, tag="s", name=f"kp{e}_{tb}")
        for sub in range(QC // CH):
            for c in range(CT):
                nc.tensor.matmul(kp[:, bass.ts(sub, CH)],
                                 wk[c][:, bass.ts(e, 128)],
                                 x2t[c][:, bass.ds(tb * QC + sub * CH, CH)],
                                 start=(c == 0), stop=(c == CT - 1))
        nc.vector.tensor_scalar_add(kt[e][:, bass.ts(tb, QC)], kp[:, 0:QC],
                                    bkr[:, e:e + 1])

    def emit_v(t):  # one key-tile of V, interleaved layout + ones col
        vp = pvp.tile([128, CH], F32, tag="vp", name=f"vp{t}")
        for c in range(CT):
            nc.tensor.matmul(vp[:, 0:EMB], x2t[c][:, bass.ts(t, 128)], wv[c][:],
                             start=(c == 0), stop=(c == CT - 1))
        eng = nc.gpsimd if V_ADD_ON_GPSIMD else nc.vector
        eng.tensor_add(
            v[:, t, :, 0:64],
            vp[:, 0:EMB].rearrange("p (h d) -> p h d", h=H),
            bvb[:].rearrange("p (h d) -> p h d", h=H))

    # Q^T[e,q] = sum_c WqT[c,e] * x1T[c,q]  (+ bq per-partition)
    for e in range(CT):
        qp = ps.tile([128, 2 * CH], F32, tag="s", name=f"qp{e}")
        for sub in range(QC // CH):
            for c in range(CT):
                nc.tensor.matmul(qp[:, bass.ts(sub, CH)],
                                 wq[c][:, bass.ts(e, 128)],
                                 x1t[c][:, bass.ts(sub, CH)],
                                 start=(c == 0), stop=(c == CT - 1))
        nc.vector.tensor_scalar_add(qt[e][:, 0:QC], qp[:, 0:QC],
                                    bqr[:, e:e + 1])

    # upfront: K^T e=0 (scores pr=0 stationary) and V key-tile 0
    for tb in range(T // QC):
        emit_k(0, tb)
    emit_v(0)
    # deferred: remaining V tiles ride the pr=0/ch=0 unit loop; K^T for
    # e=1..3 is pumped one tile per ~KT/4 units across chunks 1..3
    # (each e finished at least one full chunk before its pr needs it).
    feed = [(e, (lambda e=e, tb=tb: emit_k(e, tb)))
            for e in range(1, CT) for tb in range(T // QC)]

    # ---- attention ---------------------------------------------------
    pe_ = ctx.enter_context(tc.tile_pool(name="pe", bufs=2))
    ppt = ctx.enter_context(tc.tile_pool(name="ppt", bufs=2))
    prr = ctx.enter_context(tc.tile_pool(name="prr", bufs=2))
    py = ctx.enter_context(tc.tile_pool(name="py", bufs=1))
    po = ctx.enter_context(tc.tile_pool(name="po", bufs=2))
    yts = [py.tile([128, QC], F16, tag=f"yt{e}", name=f"yt{e}")
           for e in range(CT)]

    def emit_norm(pr, ch, av):
        # normalize: Y^T_h / r_h, r from the ones column (row 64)
        for hh in range(2):
            r0 = prr.tile([1, CH], F32, tag="r0", name="r0")
            nc.vector.tensor_copy(r0[:], av[hh][64:65, :])
            rr32 = prr.tile([1, CH], F32, tag="rr32", name="rr32")
            nc.vector.reciprocal_approx_fast(rr32[:], r0[:])
            rr = prr.tile([1, CH], F16, tag="rr", name="rr")
            with nc.allow_low_precision(reason="fp16 recip copy ok"):
                nc.vector.tensor_copy(rr[:], rr32[:])
            bc = pvp.tile([128, CH], F32, tag="vp", name=f"bc{pr}_{ch}{hh}")
            nc.tensor.matmul(bc[0:64, :], ones[:], rr[:],
                             start=True, stop=True)
            # DVE may read only one PSUM operand: stage av in SBUF fp16
            ysb = prr.tile([64, CH], F16, tag="ysb", name="ysb")
            with nc.allow_low_precision(reason="y fp16 staging ok"):
                nc.vector.tensor_copy(ysb[:], av[hh][0:64, :])
            nc.vector.tensor_mul(
                yts[pr][bass.ds(64 * hh, 64), bass.ds(ch * CH, CH)],
                ysb[:], bc[0:64, :])

    def emit_out(qi):
        pso = pvp.tile([128, CH], F32, tag="vp", name=f"pso{qi}")
        for e in range(CT):
            nc.tensor.matmul(pso[:, 0:EMB], yts[e][:, bass.ts(qi, 128)],
                             wu[e][:], start=(e == 0), stop=(e == CT - 1))
        osb = po.tile([128, EMB], F32, tag="o", name="osb")
        nc.vector.tensor_add(osb[:], pso[:, 0:EMB], bub[:])
        nc.sync.dma_start(io["out"][bass.ts(qi, 128), :], osb[:])

    pending = []             # deferred normalize of the previous chunk
    kstep = max(KT // 4, 1)  # feed pump cadence
    nchunks = PR * NCH
    for pr in range(PR):
        # safety: kt[pr] must be complete before this pr's scores
        while feed and feed[0][0] <= pr:
            feed.pop(0)[1]()
        for ch in range(NCH):
            chunk = pr * NCH + ch
            av = [pav.tile([128, CH], F32, tag="av", name=f"av{pr}_{ch}{hh}")
                  for hh in range(2)]
            for kk in range(KT):
                mask_prefetch(pr, ch, kk + 4)
                mkt = mask_ap(pr, ch, kk)
                s = ps.tile([128, 2 * CH], F32, tag="s", name="s")
                for hh in range(2):
                    nc.tensor.matmul(s[:, bass.ds(hh * CH, CH)],
                                     kt[pr][bass.ds(64 * hh, 64), bass.ts(kk, 128)],
                                     qt[pr][bass.ds(64 * hh, 64), bass.ds(ch * CH, CH)],
                                     start=True, stop=True,
                                     tile_position=(64 * hh, 0))
                # PE fills exp/mult latency with deferred work
                if chunk == 0 and kk + 1 < KT:
                    emit_v(kk + 1)
                elif 1 <= chunk <= PR * NCH - 2 and feed and kk % kstep == kstep // 2:
                    feed.pop(0)[1]()
                if chunk == nchunks - 1 and NCH > 1 and kk % 8 == 3 and kk // 8 < CH // 128:
                    emit_out(kk // 8)  # out-proj for ch0 query blocks
                e16 = pe_.tile([128, 2 * CH], F16, tag="E", name="e16")
                nc.scalar.activation(e16[:], s[:], EXP, scale=scale)
                pt = ppt.tile([128, 2 * CH], F16, tag="P", name="pt")
                nc.vector.tensor_mul(
                    pt[:].rearrange("p (h q) -> p h q", h=2),
                    e16[:].rearrange("p (h q) -> p h q", h=2),
                    mkt.unsqueeze(1).broadcast_to([128, 2, CH]))
                if kk == 0 and pending:
                    # previous chunk's normalize, emitted here so its PE
                    # work precedes AV(kk=0) (which waits on the freed
                    # accumulator slots) but follows this unit's scores
                    # - the ACT exp stream never pauses at boundaries.
                    pending.pop(0)()
                for hh in range(2):
                    nc.tensor.matmul(av[hh][0:65, :], v[:, kk, 2 * pr + hh, 0:65],
                                     pt[:, bass.ds(hh * CH, CH)],
                                     start=(kk == 0), stop=(kk == KT - 1))
            pending.append(lambda pr=pr, ch=ch, av=av: emit_norm(pr, ch, av))
    while pending:
        pending.pop(0)()
    # out-proj query blocks not already emitted inside the last chunk
    q0 = (CH // 128) if NCH > 1 else 0
    for qi in range(q0, QC // 128):
        emit_out(qi)


def build(cfg, num_devices=8):
    T, QC = cfg["T"], cfg["QC"]
    nc = bacc.Bacc("TRN2", target_bir_lowering=False, debug=False,
                   num_devices=num_devices)
    io = {
        "x1T": nc.dram_tensor("x1T", [EMB, QC], F16, kind="ExternalInput").ap(),
        "x2T": nc.dram_tensor("x2T", [EMB, T], F16, kind="ExternalInput").ap(),
        "maskT": nc.dram_tensor("maskT", [T, QC], F16, kind="ExternalInput").ap(),
        "wqT": nc.dram_tensor("wqT", [EMB, EMB], F16, kind="ExternalInput").ap(),
        "wkT": nc.dram_tensor("wkT", [EMB, EMB], F16, kind="ExternalInput").ap(),
        "wvT": nc.dram_tensor("wvT", [EMB, EMB], F16, kind="ExternalInput").ap(),
        "wuT": nc.dram_tensor("wuT", [EMB, EMB], F16, kind="ExternalInput").ap(),
        "bqr": nc.dram_tensor("bqr", [128, CT], F32, kind="ExternalInput").ap(),
        "bkr": nc.dram_tensor("bkr", [128, CT], F32, kind="ExternalInput").ap(),
        "bvb": nc.dram_tensor("bvb", [128, EMB], F32, kind="ExternalInput").ap(),
        "bub": nc.dram_tensor("bub", [128, EMB], F32, kind="ExternalInput").ap(),
        "out": nc.dram_tensor("out", [QC, EMB], F32, kind="ExternalOutput").ap(),
    }
    with tile.TileContext(nc) as tc:
        with ExitStack() as ctx:
            attention_body(ctx, tc, io, cfg)
    nc.compile()
    return nc


def host_prep(x1, x2, mask, Wq, bq, Wk, bk, Wv, bv, Wu, bu, cfg):
    """Build the 8 per-core input maps from full inputs."""
    T, QC = cfg["T"], cfg["QC"]
    shared = {
        "wqT": np.ascontiguousarray(Wq.T).astype(np.float16),
        "wkT": np.ascontiguousarray(Wk.T).astype(np.float16),
        "wvT": np.ascontiguousarray(Wv.T).astype(np.float16),
        "wuT": np.ascontiguousarray(Wu.T).astype(np.float16),
        "bqr": np.ascontiguousarray(bq.reshape(CT, 128).T).astype(np.float32),
        "bkr": np.ascontiguousarray(bk.reshape(CT, 128).T).astype(np.float32),
        "bvb": np.ascontiguousarray(np.broadcast_to(bv, (128, EMB))).astype(np.float32),
        "bub": np.ascontiguousarray(np.broadcast_to(bu, (128, EMB))).astype(np.float32),
    }
    x2T = [x2[b].T.astype(np.float16) for b in range(x1.shape[0])]
    in_maps = []
    n_cores = (x1.shape[0] * x1.shape[1]) // QC
    per_b = x1.shape[1] // QC
    for c in range(n_cores):
        b, q0 = c // per_b, (c % per_b) * QC
        in_maps.append(dict(
            shared,
            x1T=x1[b, q0:q0 + QC, :].T.astype(np.float16),
            x2T=x2T[b],
            maskT=mask[b, q0:q0 + QC, :].T.astype(np.float16),
        ))
    return in_maps


_NC_CACHE = {}


def kernel(x1, x2, mask, Wq, bq, Wk, bk, Wv, bv, Wu, bu):
    cfg = FULL_CFG
    B, TQ, _ = x1.shape
    in_maps = host_prep(np.asarray(x1, np.float32), np.asarray(x2, np.float32),
                        np.asarray(mask), np.asarray(Wq, np.float32),
                        np.asarray(bq, np.float32), np.asarray(Wk, np.float32),
                        np.asarray(bk, np.float32), np.asarray(Wv, np.float32),
                        np.asarray(bv, np.float32), np.asarray(Wu, np.float32),
                        np.asarray(bu, np.float32), cfg)
    key = (cfg["T"], cfg["QC"])
    if key not in _NC_CACHE:
        _NC_CACHE[key] = build(cfg)
    nc = _NC_CACHE[key]
    res = run_bass_kernel_spmd(nc, in_maps, core_ids=list(range(8)),
                               trace=bool(os.environ.get("KERNEL_TRACE")))
    if os.environ.get("KERNEL_TRACE"):
        kernel.last_exec_ns = res.exec_time_ns
        kernel.last_results = res
    out = np.empty((B, TQ, EMB), np.float32)
    per_b = TQ // cfg["QC"]
    for c in range(8):
        b, q0 = c // per_b, (c % per_b) * cfg["QC"]
        out[b, q0:q0 + cfg["QC"], :] = res.results[c]["out"]
    return out
